# revision 2
# baseline (speedup 1.0000x reference)
"""Trainium2 Bass kernel for AdaptedEntropyModel (vq_codebook).

Computes, for full inputs [16,192,48,64]:
  sym = searchsorted(mids, inputs - means)        (int32)
  deq = unique_values[sym] + means                (float32)
  cdf = quantized per-channel CDF table [192,66]  (int32)

Distribution: data-parallel over 8 NeuronCores along the batch dim
(2 batches/core); the tiny CDF build is replicated on every core.
"""

import sys
import types

import numpy as np

# ---------------------------------------------------------------------------
# NTFF profile hook shim: the agent image's `antenv` lacks `axon_hooks`, so
# bass_utils' trace=True path degrades. Recreate the hook from trn_boot.
# ---------------------------------------------------------------------------
def _install_ntff_shim():
    if "antenv.axon_hooks" in sys.modules:
        return
    try:
        sys.path.insert(0, "/root/.axon_site/trn_agent_boot")
        from trn_boot import _ntff_profile_via_ctypes  # type: ignore

        hook = _ntff_profile_via_ctypes("/opt/axon/libaxon_pjrt.so")
        mod = types.ModuleType("antenv.axon_hooks")
        mod.get_axon_ntff_profile_hook = lambda: hook
        mod.set_axon_ntff_profile_hook = lambda h: None
        sys.modules["antenv.axon_hooks"] = mod
        import antenv

        antenv.axon_hooks = mod
    except Exception:
        pass


_install_ntff_shim()

import concourse.bass as bass  # noqa: E402
import concourse.tile as tile  # noqa: E402
from concourse import bacc, bass_utils, mybir  # noqa: E402

AO = mybir.AluOpType
F32 = mybir.dt.float32
I32 = mybir.dt.int32

# Problem geometry (hardcoded per spec).
B, C, H, W = 16, 192, 48, 64
L = 60
MAXLEN = 64
PREC = 16
N_CORES = 8

BPC = B // N_CORES                    # batches per core
ELEMS = BPC * C * H * W               # 1_179_648 elements per core
P = 128
FREE = ELEMS // P                     # 9216
TN = 512                              # free-dim tile size
NT = FREE // TN                       # 18 tiles


def _build_nc(uv: np.ndarray):
    """Build the Bass program. uv = unique_values (float32 [60])."""
    uv = uv.astype(np.float32)
    mids = (np.float32(0.5) * (uv[1:] + uv[:-1])).astype(np.float32)  # [59]
    duv = (uv[1:] - uv[:-1]).astype(np.float32)                       # [59]
    # hinge basis for uv[s]: uv[s] = uv0 + duv0*s + sum_j gam[j]*relu(s-j)
    gam = (duv[1:] - duv[:-1]).astype(np.float32)                     # [58]

    nc = bacc.Bacc("TRN2", target_bir_lowering=False, debug=False,
                   num_devices=N_CORES)

    a_in = nc.dram_tensor("inputs", [P, FREE], F32, kind="ExternalInput").ap()
    a_mean = nc.dram_tensor("means", [P, FREE], F32, kind="ExternalInput").ap()
    a_pmf = nc.dram_tensor("pmf_pad", [C, MAXLEN + 1], F32, kind="ExternalInput").ap()
    a_tail = nc.dram_tensor("tail", [C, 1], F32, kind="ExternalInput").ap()
    a_plen = nc.dram_tensor("plen_f", [C, 1], F32, kind="ExternalInput").ap()
    a_iota = nc.dram_tensor("iota65", [P, MAXLEN + 1], F32, kind="ExternalInput").ap()

    o_sym = nc.dram_tensor("o_sym", [P, FREE], I32, kind="ExternalOutput").ap()
    o_deq = nc.dram_tensor("o_deq", [P, FREE], F32, kind="ExternalOutput").ap()
    o_cdf = nc.dram_tensor("o_cdf", [C, MAXLEN + 2], I32, kind="ExternalOutput").ap()

    with tile.TileContext(nc) as tc:
        with tc.tile_pool(name="io", bufs=3) as io_pool, \
             tc.tile_pool(name="tmp", bufs=2) as tmp_pool, \
             tc.tile_pool(name="cdf", bufs=1) as cdf_pool:

            # ---------------- CDF build (tiny; replicated on all cores) ----
            for c0, csz in ((0, 128), (128, 64)):
                t_pmf = cdf_pool.tile([csz, MAXLEN + 1], F32, tag="cpmf")
                t_tail = cdf_pool.tile([csz, 1], F32, tag="ctail")
                t_plen = cdf_pool.tile([csz, 1], F32, tag="cplen")
                t_iota = cdf_pool.tile([csz, MAXLEN + 1], F32, tag="ciota")
                nc.sync.dma_start(t_pmf[:], a_pmf[c0:c0 + csz, :])
                nc.sync.dma_start(t_tail[:], a_tail[c0:c0 + csz, :])
                nc.sync.dma_start(t_plen[:], a_plen[c0:c0 + csz, :])
                nc.sync.dma_start(t_iota[:], a_iota[0:csz, :])

                m1 = cdf_pool.tile([csz, MAXLEN + 1], F32, tag="cm1")
                prob = cdf_pool.tile([csz, MAXLEN + 1], F32, tag="cprob")
                # prob = pmf * (k < plen) + tail * (k == plen)
                nc.vector.tensor_scalar(m1[:], t_iota[:], t_plen[:, 0:1], None,
                                        op0=AO.is_lt)
                nc.vector.tensor_mul(prob[:], t_pmf[:], m1[:])
                nc.vector.tensor_scalar(m1[:], t_iota[:], t_plen[:, 0:1],
                                        t_tail[:, 0:1], op0=AO.is_equal,
                                        op1=AO.mult)
                nc.vector.tensor_add(prob[:], prob[:], m1[:])
                # cum = cumsum(prob); total = cum[:, -1]
                cum = cdf_pool.tile([csz, MAXLEN + 1], F32, tag="ccum")
                nc.vector.tensor_tensor_scan(cum[:], prob[:], prob[:], 0.0,
                                             op0=AO.add, op1=AO.bypass)
                # body = (cum * (1/total)) * 2^16 ; masked to k <= plen
                body = cdf_pool.tile([csz, MAXLEN + 1], F32, tag="cbody")
                rec = cdf_pool.tile([csz, 1], F32, tag="crec")
                nc.vector.reciprocal(rec[:], cum[:, MAXLEN:MAXLEN + 1])
                nc.vector.tensor_scalar(body[:], cum[:], rec[:, 0:1],
                                        float(1 << PREC), op0=AO.mult,
                                        op1=AO.mult)
                nc.vector.tensor_scalar(m1[:], t_iota[:], t_plen[:, 0:1], None,
                                        op0=AO.is_le)
                nc.vector.tensor_mul(body[:], body[:], m1[:])
                t_cdf = cdf_pool.tile([csz, MAXLEN + 2], I32, tag="ccdf")
                nc.vector.memset(t_cdf[:, 0:1], 0)
                nc.vector.tensor_copy(t_cdf[:, 1:MAXLEN + 2], body[:])
                nc.sync.dma_start(o_cdf[c0:c0 + csz, :], t_cdf[:])

            # ---------------- main quantize/dequantize ---------------------
            for t in range(NT):
                sl = bass.ts(t, TN)
                x_in = io_pool.tile([P, TN], F32, tag="x_in")
                m_in = io_pool.tile([P, TN], F32, tag="m_in")
                nc.sync.dma_start(x_in[:], a_in[:, sl])
                nc.sync.dma_start(m_in[:], a_mean[:, sl])

                x = tmp_pool.tile([P, TN], F32, tag="x")
                nc.vector.tensor_sub(x[:], x_in[:], m_in[:])

                # sym = sum_j (x > mids[j]) via fused compare-add
                sym = tmp_pool.tile([P, TN], F32, tag="sym")
                nc.vector.tensor_scalar(sym[:], x[:], float(mids[0]), None,
                                        op0=AO.is_gt)
                for j in range(1, 59):
                    nc.vector.scalar_tensor_tensor(
                        sym[:], x[:], float(mids[j]), sym[:],
                        op0=AO.is_gt, op1=AO.add)

                # val = uv0 + duv0*sym + sum_j gam[j]*relu(sym - j)
                val = tmp_pool.tile([P, TN], F32, tag="val")
                nc.vector.tensor_scalar(val[:], sym[:], float(duv[0]),
                                        float(uv[0]), op0=AO.mult, op1=AO.add)
                r = tmp_pool.tile([P, TN], F32, tag="r")
                for j in range(1, 59):
                    nc.vector.tensor_scalar(r[:], sym[:], float(j), 0.0,
                                            op0=AO.subtract, op1=AO.max)
                    nc.vector.scalar_tensor_tensor(
                        val[:], r[:], float(gam[j - 1]), val[:],
                        op0=AO.mult, op1=AO.add)

                # deq = val + means ; sym -> int32
                deq = io_pool.tile([P, TN], F32, tag="deq")
                nc.vector.tensor_add(deq[:], val[:], m_in[:])
                sym_i = io_pool.tile([P, TN], I32, tag="sym_i")
                nc.vector.tensor_copy(sym_i[:], sym[:])
                nc.sync.dma_start(o_sym[:, sl], sym_i[:])
                nc.sync.dma_start(o_deq[:, sl], deq[:])

    nc.compile()
    return nc


def _run(inputs, means, unique_values, pmf, tail_mass, pmf_length, trace=False):
    inputs = np.ascontiguousarray(inputs, dtype=np.float32)
    means = np.ascontiguousarray(means, dtype=np.float32)
    uv = np.asarray(unique_values, dtype=np.float32)
    pmf = np.asarray(pmf, dtype=np.float32)
    tail = np.asarray(tail_mass, dtype=np.float32).reshape(C, 1)
    plen = np.asarray(pmf_length, dtype=np.int32)

    nc = _build_nc(uv)

    pmf_pad = np.zeros((C, MAXLEN + 1), np.float32)
    pmf_pad[:, :MAXLEN] = pmf
    plen_f = plen.astype(np.float32).reshape(C, 1)
    iota65 = np.broadcast_to(
        np.arange(MAXLEN + 1, dtype=np.float32)[None, :], (P, MAXLEN + 1)
    ).copy()

    in_maps = []
    for i in range(N_CORES):
        shard_in = inputs[i * BPC:(i + 1) * BPC].reshape(P, FREE)
        shard_mean = means[i * BPC:(i + 1) * BPC].reshape(P, FREE)
        in_maps.append({
            "inputs": np.ascontiguousarray(shard_in),
            "means": np.ascontiguousarray(shard_mean),
            "pmf_pad": pmf_pad,
            "tail": tail,
            "plen_f": plen_f,
            "iota65": iota65,
        })

    res = bass_utils.run_bass_kernel_spmd(
        nc, in_maps, core_ids=list(range(N_CORES)), trace=trace)

    sym = np.concatenate(
        [res.results[i]["o_sym"].reshape(BPC, C, H, W) for i in range(N_CORES)],
        axis=0)
    deq = np.concatenate(
        [res.results[i]["o_deq"].reshape(BPC, C, H, W) for i in range(N_CORES)],
        axis=0)
    cdf = res.results[0]["o_cdf"]
    return (sym.astype(np.int32), deq.astype(np.float32),
            cdf.astype(np.int32)), res.exec_time_ns


def kernel(inputs, means, unique_values, pmf, tail_mass, pmf_length):
    out, _ = _run(inputs, means, unique_values, pmf, tail_mass, pmf_length)
    return out
